# revision 37
# baseline (speedup 1.0000x reference)
"""Multi-head self-attention (B=2, N=2048, D=1024, H=16, dh=64) on 8 trn2 cores.

Sharding: core c -> batch b = c // 4, head-group hg = c % 4 (4 heads per core).
Each core computes partial = Attn_{heads hg}(x_b) @ Wo[rows hg]; the host sums
the 4 partials per batch and adds bo (the unshard step).

Per-core pipeline (x/Wq/Wk/Wv cast to bf16 on the host; scores PSUM-exact):
  1. PE-transpose x_b -> xT bf16 (D on partitions), overlapped with the
     staged x/weight DMAs and per-block pair-0 q/k projections + v tiles
  2. attention per head pair (2 heads packed in disjoint PE row groups so the
     score matmuls can run concurrently on silicon), per query block of 512,
     per key tile of 128, software-pipelined so attn@v for tile jt-skew runs
     on the PE while ScalarE exps tile jt (skew 4 on the first block so
     attention starts while prologue projections still stream in):
       scoresT (2 bf16 MMs) -> exp on ScalarE (scale folded in, one
       [128,1024] instr covering both heads, bf16 out) -> attn@v PSUM
       accumulation with the two heads col-tiled into one [128,512] bank
       (PE col groups 0-1 / 2-3, concurrent on silicon) + softmax
       denominators as 4 concurrent [1,256] col-tiled matmuls (head x
       query-half on psum partitions 0/32/64/96)
     Pair-0's ACT-bound window is filled with v tiles + pair-1 projections;
     pair-1's window is filled with the output projection of completed query
     blocks.  Denominator reciprocals on DVE, broadcast via a DRAM bounce
     (gpsimd is broken on HW here), ctxT normalized in place on DVE.
  3. out tiles = ctxT.T @ Wo via 2-step PSUM accumulation, DVE copy, DMA out;
     bo and the 4-way head-group partial sums are applied on the host.
"""

import numpy as np

B, N, D = 2, 2048, 1024
H, DH = 16, 64
HPC = 4                # heads per core
CS = HPC * DH          # 256 = per-core slice of the inner dim
NCORES = 8
SCALE = DH ** -0.5

NT = N // 128          # 16 token tiles
KT = D // 128          # 8 contraction tiles
NIB = N // 512         # 4 query blocks

_CACHE = {}


def _build_nc(reps=1, nblocks=8):
    import concourse.bass as bass
    import concourse.bacc as bacc
    import concourse.mybir as mybir
    import concourse.tile as tile
    from contextlib import ExitStack

    f32 = mybir.dt.float32
    f32r = mybir.dt.float32r
    bf16 = mybir.dt.bfloat16
    PSUM = bass.MemorySpace.PSUM
    Exp = mybir.ActivationFunctionType.Exp

    nc = bacc.Bacc()

    x_d = nc.dram_tensor("x", [N, D], bf16, kind="ExternalInput")
    wq_d = nc.dram_tensor("wq", [D, CS], bf16, kind="ExternalInput")
    wk_d = nc.dram_tensor("wk", [D, CS], bf16, kind="ExternalInput")
    wv_d = nc.dram_tensor("wv", [D, CS], bf16, kind="ExternalInput")
    wo_d = nc.dram_tensor("wo", [CS, D], bf16, kind="ExternalInput")
    out_d = nc.dram_tensor("out", [N, D], f32, kind="ExternalOutput")
    den_d = nc.dram_tensor("den_scratch", [32, 256], f32r)

    import ml_dtypes as _mld
    ident_d = nc.inline_tensor(np.eye(128).astype(_mld.bfloat16), name="ident")

    # grouped views for batched DMAs
    x_g = x_d.rearrange("(g j p) d -> g p j d", p=128, j=2)        # [8][128,2,1024]
    wq_g = wq_d.rearrange("(k p) c -> p k c", p=128)               # [128,8,256]
    wk_g = wk_d.rearrange("(k p) c -> p k c", p=128)
    wv_g = wv_d.rearrange("(k p) c -> p k c", p=128)
    wo_g = wo_d.rearrange("(k p) c -> p k c", p=128)               # [128,2,1024]
    out_g1 = out_d.rearrange("(q p) e -> q p e", p=128)            # [16][128,1024]

    with tile.TileContext(nc) as tc:
      for _rep in range(reps):
       with ExitStack() as es:
             singles = es.enter_context(tc.tile_pool(name="singles", bufs=1))

             ones1 = singles.tile([128, 1], bf16, tag="ones1")
             nc.vector.memset(ones1, 1.0)
             # dummy activation at t=0 so the ~2.7us exp table load overlaps
             # the prologue DMAs instead of delaying the first real exp
             actwarm = singles.tile([128, 1], f32, tag="actwarm")
             nc.scalar.activation(actwarm, ones1, Exp, scale=1.0)

             wq_sb = singles.tile([128, KT, CS], bf16, tag="wq")
             wk_sb = singles.tile([128, KT, CS], bf16, tag="wk")
             wv_sb = singles.tile([128, KT, CS], bf16, tag="wv")
             wo_sb = singles.tile([128, 2, D], bf16, tag="wo")

             qT = [singles.tile([128, N], bf16, tag=f"qT{p}", name=f"qT{p}") for p in range(2)]
             kTt = [singles.tile([128, N], bf16, tag=f"kT{p}", name=f"kT{p}") for p in range(2)]
             vA = [singles.tile([128, CS], bf16, tag=f"v{t}", name=f"v{t}") for t in range(NT)]
             ctxT = [singles.tile([128, N], bf16, tag=f"ctxT{p}", name=f"ctxT{p}") for p in range(2)]
             o_sb = es.enter_context(tc.tile_pool(name="osb", bufs=2))
             rec_pool = es.enter_context(tc.tile_pool(name="recp", bufs=2))

             # ---- phase 1: transpose x; interleave pair-0 q/k and v[0:4] ----
             xps = ExitStack()
             xT_pool = xps.enter_context(tc.tile_pool(name="xTp", bufs=1))
             pj_ps = xps.enter_context(tc.tile_pool(name="pjps", bufs=2, space=PSUM))
             xT_all = xT_pool.tile([128, KT, N], bf16, tag="xT", name="xT")
             xT = [xT_all[:, k, :] for k in range(KT)]

             pending_pq = {}

             def emit_v(t, half):
                 # half 0/1: contraction tiles 0..3 / 4..7 (~430ns PE chunks
                 # so a side op never delays the next score matmul by much)
                 key = ("v", t)
                 if half == 0:
                     pv = pj_ps.tile([128, CS], f32, tag="pp", name="ppv")
                     pending_pq[key] = pv
                 else:
                     pv = pending_pq.pop(key)
                 for kk in range(4):
                     k = half * 4 + kk
                     nc.tensor.matmul(
                         pv,
                         xT[k][:, t * 128:(t + 1) * 128],
                         wv_sb[:, k, :],
                         start=(k == 0), stop=(k == KT - 1),
                     )
                 if half == 1:
                     nc.vector.tensor_copy(vA[t], pv)

             def emit_qk(dst, w_sb, p, ib, quar):
                 # quarter 0..3: contraction tiles 2q..2q+1 (~430ns PE chunks;
                 # all four accumulate into one psum tile)
                 key = (id(dst), p, ib)
                 if quar == 0:
                     pq = pj_ps.tile([128, 512], f32, tag="pp", name="pp")
                     pending_pq[key] = pq
                 else:
                     pq = pending_pq[key]
                 for kk in range(2):
                     k = 2 * quar + kk
                     nc.tensor.matmul(
                         pq,
                         w_sb[:, k, p * 128:(p + 1) * 128],
                         xT[k][:, ib * 512:(ib + 1) * 512],
                         start=(k == 0), stop=(k == KT - 1),
                     )
                 if quar == 3:
                     pending_pq.pop(key)
                     nc.vector.tensor_copy(dst[p][:, ib * 512:(ib + 1) * 512], pq)

             with ExitStack() as pes:
                 idp = pes.enter_context(tc.tile_pool(name="idp", bufs=1))
                 x_pool = pes.enter_context(tc.tile_pool(name="xp", bufs=2))
                 tp_ps = pes.enter_context(tc.tile_pool(name="tpps", bufs=2, space=PSUM))

                 ident = idp.tile([128, 128], bf16, tag="ident")
                 nc.sync.dma_start(out=ident, in_=ident_d[:, :])


                 wdma = {
                     0: lambda: nc.sync.dma_start(out=wq_sb, in_=wq_g),
                     1: lambda: nc.sync.dma_start(out=wk_sb, in_=wk_g),
                     2: lambda: nc.sync.dma_start(out=wv_sb, in_=wv_g),
                     6: lambda: nc.sync.dma_start(out=wo_sb, in_=wo_g),
                 }
                 # pair-0 projection/v emissions paced so each slot's weight
                 # DMA (fired at g=0..2) has landed well before first use
                 def _qk4(dst, w, p, ib):
                     return [
                         (lambda q=q: emit_qk(dst, w, p, ib, q)) for q in range(4)
                     ]
                 prologue_emits = {
                     1: _qk4(qT, wq_sb, 0, 0),
                     2: _qk4(kTt, wk_sb, 0, 0),
                     3: _qk4(qT, wq_sb, 0, 1),
                     4: _qk4(kTt, wk_sb, 0, 1),
                     5: _qk4(qT, wq_sb, 0, 2) + [
                         lambda: emit_v(0, 0), lambda: emit_v(0, 1)],
                     6: [lambda: emit_v(1, 0), lambda: emit_v(1, 1)],
                 }
                 for g in range(NT // 2):  # 8 groups of 2 token tiles
                     xt = x_pool.tile([128, 2, D], bf16, tag="x", name="xt")
                     if g == 0:
                         nc.sync.dma_start(out=xt[:, 0, :], in_=x_g[g][:, 0, :])
                         nc.sync.dma_start(out=xt[:, 1, :], in_=x_g[g][:, 1, :])
                     else:
                         nc.sync.dma_start(out=xt, in_=x_g[g])
                     if g in wdma:
                         wdma[g]()
                     for dh in range(KT // 4):
                         ps = tp_ps.tile([128, 4, 256], bf16, tag="tp", name="tp")
                         for dj in range(4):
                             d = 4 * dh + dj
                             for j in range(2):
                                 nc.tensor.transpose(
                                     ps[:, dj, j * 128:(j + 1) * 128],
                                     xt[:, j, d * 128:(d + 1) * 128],
                                     ident,
                                 )
                         nc.vector.tensor_copy(
                             xT_all[:, 4 * dh:4 * dh + 4, g * 256:(g + 1) * 256],
                             ps,
                         )
                     for op in prologue_emits.get(g, []):
                         op()

             # side work queue.  ORDER AND PACING ARE CORRECTNESS-CRITICAL:
             # the tile framework tracks dependencies in emission order, so
             # each chunk's output copy must be EMITTED before the first
             # attention instruction that reads it.  p0-ib0 scores read kT
             # tile jt at iter jt; av(jt) (skew 4) reads vA[jt] at iter
             # jt+4, and the post-loop drain needs ALL of vA by iter 15.
             # Pacing 3/iter for iters 0-3 then 2/iter puts position P at
             # iter P/3 (P<=12) else 4+(P-13)/2: kT-ib3 copy @2 (< jt12),
             # vA[t] copy @t exactly (pops precede the av() emission within
             # an iter, so same-iter is safe).
             side_ops = _qk4(kTt, wk_sb, 0, 2) + _qk4(kTt, wk_sb, 0, 3)
             for t in range(2, NT):
                 side_ops += [lambda t=t: emit_v(t, 0), lambda t=t: emit_v(t, 1)]
             side_ops += _qk4(qT, wq_sb, 0, 3)        # q ib3: needed @48
             for ib in range(NIB):                    # pair-1: needed @64+
                 side_ops += _qk4(qT, wq_sb, 1, ib) + _qk4(kTt, wk_sb, 1, ib)

             def emit_out(it, eh):
                 # out tile [128 tok, 512 D] for token tile `it`, D half `eh`
                 ehs = slice(eh * 512, (eh + 1) * 512)
                 po = pj_ps.tile([128, 512], f32, tag="pp", name="po")
                 for cp in range(2):
                     nc.tensor.matmul(
                         po,
                         ctxT[cp][:, it * 128:(it + 1) * 128],
                         wo_sb[:, cp, ehs],
                         start=(cp == 0), stop=(cp == 1),
                     )
                 ot = ot_tiles[it]
                 nc.vector.tensor_copy(ot[:, ehs], po)

             ot_tiles = {}

             def flush_out(q):
                 nc.sync.dma_start(out=out_g1[q], in_=ot_tiles[q])

             # ---- attention ----
             with ExitStack() as aes:
                 sc_ps = aes.enter_context(tc.tile_pool(name="scps", bufs=2, space=PSUM))
                 cb_ps = aes.enter_context(tc.tile_pool(name="cbps", bufs=1, space=PSUM))
                 dn_ps = aes.enter_context(tc.tile_pool(name="dnps", bufs=1, space=PSUM))
                 exp_pool = aes.enter_context(tc.tile_pool(name="expp", bufs=6))
                 bc_pool = aes.enter_context(tc.tile_pool(name="bcp", bufs=2))

                 out_q = []  # deferred out-projection ops (window 1)

                 if nblocks < 8:  # timing probe: truncate cleanly
                     side_ops[:] = side_ops[:{2: 40, 5: 72}.get(nblocks, 72)]
                 nb_done = 0
                 for p in range(2):
                     lh0, lh1 = 2 * p, 2 * p + 1
                     for ib in range(NIB):
                         if nb_done >= nblocks:
                             continue
                         nb_done += 1
                         ibs = slice(ib * 512, (ib + 1) * 512)
                         # both heads' ctx share one PSUM bank: head0 on
                         # partitions 0:64 (PE col groups 0-1), head1 on
                         # 64:128 (groups 2-3) -> the two attn@v matmuls
                         # run concurrently on disjoint col groups
                         cb = cb_ps.tile([128, 512], f32, tag="cb", name="cb")
                         # softmax denominators: 4 concurrent [1,256]
                         # col-tiled matmuls per key tile (head x query-half
                         # on partitions 0/32/64/96)
                         dn = dn_ps.tile([128, 256], f32, tag="dn", name="dn")

                         def av(jt, e):
                             nc.tensor.matmul(
                                 cb[0:64, :],
                                 vA[jt][:, lh0 * DH:(lh0 + 1) * DH],
                                 e[:, 0, :],
                                 start=(jt == 0), stop=(jt == NT - 1),
                             )
                             nc.tensor.matmul(
                                 cb[64:128, :],
                                 vA[jt][:, lh1 * DH:(lh1 + 1) * DH],
                                 e[:, 1, :],
                                 start=(jt == 0), stop=(jt == NT - 1),
                             )
                             for s in range(4):
                                 h, qh = s >> 1, s & 1
                                 nc.tensor.matmul(
                                     dn[32 * s:32 * s + 1, :],
                                     ones1,
                                     e[:, h, qh * 256:(qh + 1) * 256],
                                     start=(jt == 0), stop=(jt == NT - 1),
                                     tile_position=(0, 32 * s),
                                 )

                         # software-pipelined: av lags exp by `skew` tiles
                         # (deep skew on the first block so attention can
                         # start while prologue projections still stream in)
                         skew = 4 if (p == 0 and ib == 0) else 1
                         fed = 0
                         pend = []
                         for jt in range(NT):
                             js = slice(jt * 128, (jt + 1) * 128)
                             sc = sc_ps.tile([128, 2, 512], f32, tag="sc", name="sc")
                             nc.tensor.matmul(
                                 sc[:, 0, :],
                                 kTt[p][0:64, js],
                                 qT[p][0:64, ibs],
                                 start=True, stop=True,
                             )
                             nc.tensor.matmul(
                                 sc[:, 1, :],
                                 kTt[p][64:128, js],
                                 qT[p][64:128, ibs],
                                 start=True, stop=True,
                             )
                             # pace fill work into the ACT-bound pipeline:
                             # ~1 chunk (~430ns PE) per jt step so the queued
                             # side work never delays the next score matmuls
                             # past the ACT period; early chunks (k/q tails,
                             # v tiles) must stay ahead of their consumers
                             if p == 0:
                                 if ib == 0:
                                     n = 3 if fed < 12 else 2
                                 else:
                                     n = 1
                                 for _ in range(n):
                                     if side_ops:
                                         side_ops.pop(0)()
                                         fed += 1
                             else:
                                 if side_ops:
                                     side_ops.pop(0)()
                                 else:
                                     budget = 2 if ib == NIB - 1 else 1
                                     while out_q and (budget > 0 or out_q[0][0] == 0):
                                         c, f = out_q.pop(0)
                                         f()
                                         budget -= c
                             e = exp_pool.tile([128, 2, 512], bf16, tag="exp", name="exp")
                             nc.scalar.activation(
                                 e.rearrange("p a b -> p (a b)"),
                                 sc.rearrange("p a b -> p (a b)"),
                                 Exp, scale=SCALE,
                             )
                             pend.append((jt, e))
                             if len(pend) > skew:
                                 av(*pend.pop(0))
                         for t in pend:
                             av(*t)
                         # flush: recips of the 4 denom rows in one strided
                         # DVE op (lanes 0/32/64/96), copy unnormalized ctx,
                         # broadcast recips via a DRAM bounce (gpsimd is
                         # broken on HW here), normalize ctxT in place
                         # flush: recips of the 4 denom rows in one DVE op
                         # (full-partition — lanes other than 0/32/64/96
                         # compute garbage that is never read; strided
                         # partition APs are illegal on DVE), broadcast via
                         # a DRAM bounce (gpsimd and PE row-tiled rank-1
                         # broadcasts are both broken here), normalize ctxT
                         rec = rec_pool.tile([128, 256], f32r, tag="rec", name="rec")
                         with nc.allow_low_precision(reason="f32r softmax denom"):
                             nc.vector.reciprocal(rec, dn)
                         nc.vector.tensor_copy(ctxT[p][:, ibs], cb)
                         ri = 4 * p + ib
                         for s in range(4):
                             nc.sync.dma_start(
                                 out=den_d[4*ri+s:4*ri+s+1, :],
                                 in_=rec[32*s:32*s+1, :])
                         bc = bc_pool.tile([128, 512], f32r, tag="bc", name="bc")
                         for qh in range(2):
                             for h in range(2):
                                 s = (h << 1) | qh
                                 nc.sync.dma_start(
                                     out=bc[64*h:64*h+64, 256*qh:256*qh+256],
                                     in_=den_d[4*ri+s:4*ri+s+1, :].to_broadcast((64, 256)))
                             qs = slice(ib * 512 + qh * 256, ib * 512 + qh * 256 + 256)
                             nc.vector.tensor_mul(
                                 ctxT[p][:, qs], ctxT[p][:, qs],
                                 bc[:, 256*qh:256*qh+256])
                         # queue this block's output projection for window 1
                         # (needs both pairs' ctxT for these tokens)
                         if p == 0:
                             continue
                         for itl in range(4 * ib, 4 * ib + 4):
                             def mk(itl=itl):
                                 ot_tiles[itl] = o_sb.tile(
                                     [128, D], f32, tag="ot", name="ot"
                                 )
                             out_q.append((0, mk))
                             for eh in range(2):
                                 out_q.append(
                                     (1, lambda itl=itl, eh=eh: emit_out(itl, eh)))
                             out_q.append((0, lambda itl=itl: flush_out(itl)))
                     if p == 0:
                         while side_ops:
                             side_ops.pop(0)()

                 if nblocks < 8:  # probe: anchor liveness of the chain
                     nc.sync.dma_start(
                         out=out_g1[15], in_=ctxT[0][:, 0:2048].bitcast(f32))
                 # tail: drain remaining output projection work
                 while out_q:
                     out_q.pop(0)[1]()

             xps.close()

    nc.compile()
    return nc


def get_nc():
    if "nc" not in _CACHE:
        _CACHE["nc"] = _build_nc()
    return _CACHE["nc"]


def make_in_maps(x, Wq, Wk, Wv, Wo, bo):
    import ml_dtypes
    bf = ml_dtypes.bfloat16
    x = np.ascontiguousarray(np.asarray(x, dtype=np.float32).astype(bf))
    Wq = np.asarray(Wq, dtype=np.float32).astype(bf)
    Wk = np.asarray(Wk, dtype=np.float32).astype(bf)
    Wv = np.asarray(Wv, dtype=np.float32).astype(bf)
    Wo = np.asarray(Wo, dtype=np.float32)
    in_maps = []
    for c in range(NCORES):
        b, hg = c // 4, c % 4
        sl = slice(hg * CS, (hg + 1) * CS)
        in_maps.append({
            "x": x[b],
            "wq": np.ascontiguousarray(Wq[:, sl]),
            "wk": np.ascontiguousarray(Wk[:, sl]),
            "wv": np.ascontiguousarray(Wv[:, sl]),
            "wo": np.ascontiguousarray(Wo[sl, :]).astype(bf),
        })
    return in_maps


def combine_outputs(results, bo):
    outs = [np.asarray(r["out"], dtype=np.float64) for r in results]
    full = np.stack([
        outs[0] + outs[1] + outs[2] + outs[3],
        outs[4] + outs[5] + outs[6] + outs[7],
    ]) + np.asarray(bo, dtype=np.float64)
    return full.astype(np.float32)


def kernel(x, Wq, Wk, Wv, Wo, bo):
    from concourse.bass_utils import run_bass_kernel_spmd

    nc = get_nc()
    in_maps = make_in_maps(x, Wq, Wk, Wv, Wo, bo)
    res = run_bass_kernel_spmd(nc, in_maps, list(range(NCORES)))
    return combine_outputs(res.results, bo)



# revision 38
# speedup vs baseline: 1.1566x; 1.1566x over previous
"""Multi-head self-attention (B=2, N=2048, D=1024, H=16, dh=64) on 8 trn2 cores.

Sharding: core c -> batch b = c // 4, head-group hg = c % 4 (4 heads per core).
Each core computes partial = Attn_{heads hg}(x_b) @ Wo[rows hg]; the host sums
the 4 partials per batch and adds bo (the unshard step).

Per-core pipeline (x/Wq/Wk/Wv cast to bf16 on the host; scores PSUM-exact):
  1. PE-transpose x_b -> xT bf16 (D on partitions), overlapped with the
     staged x/weight DMAs and per-block pair-0 q/k projections + v tiles
  2. attention per head pair (2 heads packed in disjoint PE row groups so the
     score matmuls can run concurrently on silicon), per query block of 512,
     per key tile of 128, software-pipelined so attn@v for tile jt-skew runs
     on the PE while ScalarE exps tile jt (skew 4 on the first block so
     attention starts while prologue projections still stream in):
       scoresT (2 bf16 MMs) -> exp on ScalarE (scale folded in, one
       [128,1024] instr covering both heads, bf16 out) -> attn@v PSUM
       accumulation with the two heads col-tiled into one [128,512] bank
       (PE col groups 0-1 / 2-3, concurrent on silicon) + softmax
       denominators as 4 concurrent [1,256] col-tiled matmuls (head x
       query-half on psum partitions 0/32/64/96)
     Pair-0's ACT-bound window is filled with v tiles + pair-1 projections;
     pair-1's window is filled with the output projection of completed query
     blocks.  Denominator reciprocals on DVE, broadcast via a DRAM bounce
     (gpsimd is broken on HW here), ctxT normalized in place on DVE.
  3. out tiles = ctxT.T @ Wo via 2-step PSUM accumulation, DVE copy, DMA out;
     bo and the 4-way head-group partial sums are applied on the host.
"""

import numpy as np

B, N, D = 2, 2048, 1024
H, DH = 16, 64
HPC = 4                # heads per core
CS = HPC * DH          # 256 = per-core slice of the inner dim
NCORES = 8
SCALE = DH ** -0.5

NT = N // 128          # 16 token tiles
KT = D // 128          # 8 contraction tiles
NIB = N // 512         # 4 query blocks

_CACHE = {}


def _build_nc(reps=1, nblocks=8):
    import concourse.bass as bass
    import concourse.bacc as bacc
    import concourse.mybir as mybir
    import concourse.tile as tile
    from contextlib import ExitStack

    f32 = mybir.dt.float32
    f32r = mybir.dt.float32r
    bf16 = mybir.dt.bfloat16
    PSUM = bass.MemorySpace.PSUM
    Exp = mybir.ActivationFunctionType.Exp

    nc = bacc.Bacc()

    x_d = nc.dram_tensor("x", [N, D], bf16, kind="ExternalInput")
    wq_d = nc.dram_tensor("wq", [D, CS], bf16, kind="ExternalInput")
    wk_d = nc.dram_tensor("wk", [D, CS], bf16, kind="ExternalInput")
    wv_d = nc.dram_tensor("wv", [D, CS], bf16, kind="ExternalInput")
    wo_d = nc.dram_tensor("wo", [CS, D], bf16, kind="ExternalInput")
    out_d = nc.dram_tensor("out", [N, D], f32, kind="ExternalOutput")
    den_d = nc.dram_tensor("den_scratch", [32, 256], f32r)

    import ml_dtypes as _mld
    ident_d = nc.inline_tensor(np.eye(128).astype(_mld.bfloat16), name="ident")

    # grouped views for batched DMAs
    x_g = x_d.rearrange("(g j p) d -> g p j d", p=128, j=2)        # [8][128,2,1024]
    wq_g = wq_d.rearrange("(k p) c -> p k c", p=128)               # [128,8,256]
    wk_g = wk_d.rearrange("(k p) c -> p k c", p=128)
    wv_g = wv_d.rearrange("(k p) c -> p k c", p=128)
    wo_g = wo_d.rearrange("(k p) c -> p k c", p=128)               # [128,2,1024]
    out_g1 = out_d.rearrange("(q p) e -> q p e", p=128)            # [16][128,1024]

    with tile.TileContext(nc) as tc:
      for _rep in range(reps):
       with ExitStack() as es:
             singles = es.enter_context(tc.tile_pool(name="singles", bufs=1))

             ones1 = singles.tile([128, 1], bf16, tag="ones1")
             nc.vector.memset(ones1, 1.0)
             # dummy activation at t=0 so the ~2.7us exp table load overlaps
             # the prologue DMAs instead of delaying the first real exp
             actwarm = singles.tile([128, 1], f32, tag="actwarm")
             nc.scalar.activation(actwarm, ones1, Exp, scale=1.0)

             wq_sb = singles.tile([128, KT, CS], bf16, tag="wq")
             wk_sb = singles.tile([128, KT, CS], bf16, tag="wk")
             wv_sb = singles.tile([128, KT, CS], bf16, tag="wv")
             wo_sb = singles.tile([128, 2, D], bf16, tag="wo")

             qT = [singles.tile([128, N], bf16, tag=f"qT{p}", name=f"qT{p}") for p in range(2)]
             kTt = [singles.tile([128, N], bf16, tag=f"kT{p}", name=f"kT{p}") for p in range(2)]
             vA = [singles.tile([128, CS], bf16, tag=f"v{t}", name=f"v{t}") for t in range(NT)]
             ctxT = [singles.tile([128, N], bf16, tag=f"ctxT{p}", name=f"ctxT{p}") for p in range(2)]
             o_sb = es.enter_context(tc.tile_pool(name="osb", bufs=2))
             rec_pool = es.enter_context(tc.tile_pool(name="recp", bufs=2))

             # ---- phase 1: transpose x; interleave pair-0 q/k and v[0:4] ----
             xps = ExitStack()
             xT_pool = xps.enter_context(tc.tile_pool(name="xTp", bufs=1))
             pj_ps = xps.enter_context(tc.tile_pool(name="pjps", bufs=2, space=PSUM))
             xT_all = xT_pool.tile([128, KT, N], bf16, tag="xT", name="xT")
             xT = [xT_all[:, k, :] for k in range(KT)]

             pending_pq = {}

             def emit_v(t, half):
                 # half 0/1: contraction tiles 0..3 / 4..7 (~430ns PE chunks
                 # so a side op never delays the next score matmul by much)
                 key = ("v", t)
                 if half == 0:
                     pv = pj_ps.tile([128, CS], f32, tag="pp", name="ppv")
                     pending_pq[key] = pv
                 else:
                     pv = pending_pq.pop(key)
                 for kk in range(4):
                     k = half * 4 + kk
                     nc.tensor.matmul(
                         pv,
                         xT[k][:, t * 128:(t + 1) * 128],
                         wv_sb[:, k, :],
                         start=(k == 0), stop=(k == KT - 1),
                     )
                 if half == 1:
                     nc.vector.tensor_copy(vA[t], pv)

             def emit_qk(dst, w_sb, p, ib, quar):
                 # quarter 0..3: contraction tiles 2q..2q+1 (~430ns PE chunks;
                 # all four accumulate into one psum tile)
                 key = (id(dst), p, ib)
                 if quar == 0:
                     pq = pj_ps.tile([128, 512], f32, tag="pp", name="pp")
                     pending_pq[key] = pq
                 else:
                     pq = pending_pq[key]
                 for kk in range(2):
                     k = 2 * quar + kk
                     nc.tensor.matmul(
                         pq,
                         w_sb[:, k, p * 128:(p + 1) * 128],
                         xT[k][:, ib * 512:(ib + 1) * 512],
                         start=(k == 0), stop=(k == KT - 1),
                     )
                 if quar == 3:
                     pending_pq.pop(key)
                     nc.vector.tensor_copy(dst[p][:, ib * 512:(ib + 1) * 512], pq)

             with ExitStack() as pes:
                 idp = pes.enter_context(tc.tile_pool(name="idp", bufs=1))
                 x_pool = pes.enter_context(tc.tile_pool(name="xp", bufs=2))
                 tp_ps = pes.enter_context(tc.tile_pool(name="tpps", bufs=2, space=PSUM))

                 ident = idp.tile([128, 128], bf16, tag="ident")
                 nc.sync.dma_start(out=ident, in_=ident_d[:, :])


                 wdma = {
                     0: lambda: nc.sync.dma_start(out=wq_sb, in_=wq_g),
                     1: lambda: nc.sync.dma_start(out=wk_sb, in_=wk_g),
                     2: lambda: nc.sync.dma_start(out=wv_sb, in_=wv_g),
                     6: lambda: nc.sync.dma_start(out=wo_sb, in_=wo_g),
                 }
                 # pair-0 projection/v emissions paced so each slot's weight
                 # DMA (fired at g=0..2) has landed well before first use
                 def _qk4(dst, w, p, ib):
                     return [
                         (lambda q=q: emit_qk(dst, w, p, ib, q)) for q in range(4)
                     ]
                 prologue_emits = {
                     1: _qk4(qT, wq_sb, 0, 0),
                     2: _qk4(kTt, wk_sb, 0, 0),
                     3: _qk4(qT, wq_sb, 0, 1),
                     4: _qk4(kTt, wk_sb, 0, 1),
                     5: _qk4(qT, wq_sb, 0, 2) + [
                         lambda: emit_v(0, 0), lambda: emit_v(0, 1)],
                     6: [lambda: emit_v(1, 0), lambda: emit_v(1, 1)],
                 }
                 for g in range(NT // 2):  # 8 groups of 2 token tiles
                     xt = x_pool.tile([128, 2, D], bf16, tag="x", name="xt")
                     if g == 0:
                         nc.sync.dma_start(out=xt[:, 0, :], in_=x_g[g][:, 0, :])
                         nc.sync.dma_start(out=xt[:, 1, :], in_=x_g[g][:, 1, :])
                     else:
                         nc.sync.dma_start(out=xt, in_=x_g[g])
                     if g in wdma:
                         wdma[g]()
                     for dh in range(KT // 4):
                         ps = tp_ps.tile([128, 4, 256], bf16, tag="tp", name="tp")
                         for dj in range(4):
                             d = 4 * dh + dj
                             for j in range(2):
                                 nc.tensor.transpose(
                                     ps[:, dj, j * 128:(j + 1) * 128],
                                     xt[:, j, d * 128:(d + 1) * 128],
                                     ident,
                                 )
                         nc.vector.tensor_copy(
                             xT_all[:, 4 * dh:4 * dh + 4, g * 256:(g + 1) * 256],
                             ps,
                         )
                     for op in prologue_emits.get(g, []):
                         op()

             # side work queue.  ORDER AND PACING ARE CORRECTNESS-CRITICAL:
             # the tile framework tracks dependencies in emission order, so
             # each chunk's output copy must be EMITTED before the first
             # attention instruction that reads it.  p0-ib0 scores read kT
             # tile jt at iter jt; av(jt) (skew 4) reads vA[jt] at iter
             # jt+4, and the post-loop drain needs ALL of vA by iter 15.
             # Pacing 3/iter for iters 0-3 then 2/iter puts position P at
             # iter P/3 (P<=12) else 4+(P-13)/2: kT-ib3 copy @2 (< jt12),
             # vA[t] copy @t exactly (pops precede the av() emission within
             # an iter, so same-iter is safe).
             side_ops = _qk4(kTt, wk_sb, 0, 2) + _qk4(kTt, wk_sb, 0, 3)
             for t in range(2, NT):
                 side_ops += [lambda t=t: emit_v(t, 0), lambda t=t: emit_v(t, 1)]
             side_ops += _qk4(qT, wq_sb, 0, 3)        # q ib3: needed @48
             for ib in range(NIB):                    # pair-1: needed @64+
                 side_ops += _qk4(qT, wq_sb, 1, ib) + _qk4(kTt, wk_sb, 1, ib)

             def emit_out(it, eh):
                 # out tile [128 tok, 512 D] for token tile `it`, D half `eh`
                 ehs = slice(eh * 512, (eh + 1) * 512)
                 po = pj_ps.tile([128, 512], f32, tag="pp", name="po")
                 for cp in range(2):
                     nc.tensor.matmul(
                         po,
                         ctxT[cp][:, it * 128:(it + 1) * 128],
                         wo_sb[:, cp, ehs],
                         start=(cp == 0), stop=(cp == 1),
                     )
                 ot = ot_tiles[it]
                 nc.vector.tensor_copy(ot[:, ehs], po)

             ot_tiles = {}

             def flush_out(q):
                 nc.sync.dma_start(out=out_g1[q], in_=ot_tiles[q])

             # ---- attention ----
             with ExitStack() as aes:
                 sc_ps = aes.enter_context(tc.tile_pool(name="scps", bufs=2, space=PSUM))
                 cb_ps = aes.enter_context(tc.tile_pool(name="cbps", bufs=1, space=PSUM))
                 dn_ps = aes.enter_context(tc.tile_pool(name="dnps", bufs=1, space=PSUM))
                 exp_pool = aes.enter_context(tc.tile_pool(name="expp", bufs=6))
                 bc_pool = aes.enter_context(tc.tile_pool(name="bcp", bufs=2))

                 out_q = []  # deferred out-projection ops (window 1)

                 if nblocks < 8:  # timing probe: truncate cleanly
                     side_ops[:] = side_ops[:{2: 40, 5: 72}.get(nblocks, 72)]
                 nb_done = 0
                 for p in range(2):
                     lh0, lh1 = 2 * p, 2 * p + 1
                     for ib in range(NIB):
                         if nb_done >= nblocks:
                             continue
                         nb_done += 1
                         ibs = slice(ib * 512, (ib + 1) * 512)
                         # both heads' ctx share one PSUM bank: head0 on
                         # partitions 0:64 (PE col groups 0-1), head1 on
                         # 64:128 (groups 2-3) -> the two attn@v matmuls
                         # run concurrently on disjoint col groups
                         cb = cb_ps.tile([128, 512], f32, tag="cb", name="cb")
                         # softmax denominators: 4 concurrent [1,256]
                         # col-tiled matmuls per key tile (head x query-half
                         # on partitions 0/32/64/96)
                         dn = dn_ps.tile([128, 256], f32, tag="dn", name="dn")

                         def av(jt, e):
                             nc.tensor.matmul(
                                 cb[0:64, :],
                                 vA[jt][:, lh0 * DH:(lh0 + 1) * DH],
                                 e[:, 0, :],
                                 start=(jt == 0), stop=(jt == NT - 1),
                             )
                             nc.tensor.matmul(
                                 cb[64:128, :],
                                 vA[jt][:, lh1 * DH:(lh1 + 1) * DH],
                                 e[:, 1, :],
                                 start=(jt == 0), stop=(jt == NT - 1),
                             )
                             for s in range(4):
                                 h, qh = s >> 1, s & 1
                                 nc.tensor.matmul(
                                     dn[32 * s:32 * s + 1, :],
                                     ones1,
                                     e[:, h, qh * 256:(qh + 1) * 256],
                                     start=(jt == 0), stop=(jt == NT - 1),
                                     tile_position=(0, 32 * s),
                                 )

                         # software-pipelined: av lags exp by `skew` tiles
                         # (deep skew on the first block so attention can
                         # start while prologue projections still stream in)
                         skew = 4 if (p == 0 and ib == 0) else 1
                         fed = 0
                         pend = []
                         for jt in range(NT):
                             js = slice(jt * 128, (jt + 1) * 128)
                             sc = sc_ps.tile([128, 2, 512], f32, tag="sc", name="sc")
                             nc.tensor.matmul(
                                 sc[:, 0, :],
                                 kTt[p][0:64, js],
                                 qT[p][0:64, ibs],
                                 start=True, stop=True,
                             )
                             nc.tensor.matmul(
                                 sc[:, 1, :],
                                 kTt[p][64:128, js],
                                 qT[p][64:128, ibs],
                                 start=True, stop=True,
                             )
                             # pace fill work into the ACT-bound pipeline:
                             # ~1 chunk (~430ns PE) per jt step so the queued
                             # side work never delays the next score matmuls
                             # past the ACT period; early chunks (k/q tails,
                             # v tiles) must stay ahead of their consumers
                             if p == 0:
                                 if ib == 0:
                                     n = 3 if fed < 12 else 2
                                 else:
                                     n = 1
                                 for _ in range(n):
                                     if side_ops:
                                         side_ops.pop(0)()
                                         fed += 1
                             else:
                                 if side_ops:
                                     side_ops.pop(0)()
                                 else:
                                     budget = 2 if ib == NIB - 1 else 1
                                     while out_q and (budget > 0 or out_q[0][0] == 0):
                                         c, f = out_q.pop(0)
                                         f()
                                         budget -= c
                             e = exp_pool.tile([128, 2, 512], bf16, tag="exp", name="exp")
                             nc.scalar.activation(
                                 e.rearrange("p a b -> p (a b)"),
                                 sc.rearrange("p a b -> p (a b)"),
                                 Exp, scale=SCALE,
                             )
                             pend.append((jt, e))
                             if len(pend) > skew:
                                 av(*pend.pop(0))
                         for t in pend:
                             av(*t)
                         # flush: recips of the 4 denom rows in one strided
                         # DVE op (lanes 0/32/64/96), copy unnormalized ctx,
                         # broadcast recips via a DRAM bounce (gpsimd is
                         # broken on HW here), normalize ctxT in place
                         # flush: recips of the 4 denom rows in one DVE op
                         # (full-partition — lanes other than 0/32/64/96
                         # compute garbage that is never read; strided
                         # partition APs are illegal on DVE), broadcast via
                         # a DRAM bounce (gpsimd and PE row-tiled rank-1
                         # broadcasts are both broken here), normalize ctxT
                         rec = rec_pool.tile([128, 256], f32r, tag="rec", name="rec")
                         with nc.allow_low_precision(reason="f32r softmax denom"):
                             nc.vector.reciprocal(rec, dn)
                         nc.vector.tensor_copy(ctxT[p][:, ibs], cb)
                         ri = 4 * p + ib
                         for s in range(4):
                             nc.sync.dma_start(
                                 out=den_d[4*ri+s:4*ri+s+1, :],
                                 in_=rec[32*s:32*s+1, :])
                         bc = bc_pool.tile([128, 512], f32r, tag="bc", name="bc")
                         for s in range(4):
                             h, qh = s >> 1, s & 1
                             nc.sync.dma_start(
                                 out=bc[64*h:64*h+64, 256*qh:256*qh+256],
                                 in_=den_d[4*ri+s:4*ri+s+1, :].to_broadcast((64, 256)))
                         nc.vector.tensor_mul(ctxT[p][:, ibs], ctxT[p][:, ibs], bc)
                         # queue this block's output projection for window 1
                         # (needs both pairs' ctxT for these tokens)
                         if p == 0:
                             continue
                         for itl in range(4 * ib, 4 * ib + 4):
                             def mk(itl=itl):
                                 ot_tiles[itl] = o_sb.tile(
                                     [128, D], f32, tag="ot", name="ot"
                                 )
                             out_q.append((0, mk))
                             for eh in range(2):
                                 out_q.append(
                                     (1, lambda itl=itl, eh=eh: emit_out(itl, eh)))
                             out_q.append((0, lambda itl=itl: flush_out(itl)))
                     if p == 0:
                         while side_ops:
                             side_ops.pop(0)()

                 if nblocks < 8:  # probe: anchor liveness of the chain
                     nc.sync.dma_start(
                         out=out_g1[15], in_=ctxT[0][:, 0:2048].bitcast(f32))
                 # tail: drain remaining output projection work
                 while out_q:
                     out_q.pop(0)[1]()

             xps.close()

    nc.compile()
    return nc


def get_nc():
    if "nc" not in _CACHE:
        _CACHE["nc"] = _build_nc()
    return _CACHE["nc"]


def make_in_maps(x, Wq, Wk, Wv, Wo, bo):
    import ml_dtypes
    bf = ml_dtypes.bfloat16
    x = np.ascontiguousarray(np.asarray(x, dtype=np.float32).astype(bf))
    Wq = np.asarray(Wq, dtype=np.float32).astype(bf)
    Wk = np.asarray(Wk, dtype=np.float32).astype(bf)
    Wv = np.asarray(Wv, dtype=np.float32).astype(bf)
    Wo = np.asarray(Wo, dtype=np.float32)
    in_maps = []
    for c in range(NCORES):
        b, hg = c // 4, c % 4
        sl = slice(hg * CS, (hg + 1) * CS)
        in_maps.append({
            "x": x[b],
            "wq": np.ascontiguousarray(Wq[:, sl]),
            "wk": np.ascontiguousarray(Wk[:, sl]),
            "wv": np.ascontiguousarray(Wv[:, sl]),
            "wo": np.ascontiguousarray(Wo[sl, :]).astype(bf),
        })
    return in_maps


def combine_outputs(results, bo):
    outs = [np.asarray(r["out"], dtype=np.float64) for r in results]
    full = np.stack([
        outs[0] + outs[1] + outs[2] + outs[3],
        outs[4] + outs[5] + outs[6] + outs[7],
    ]) + np.asarray(bo, dtype=np.float64)
    return full.astype(np.float32)


def kernel(x, Wq, Wk, Wv, Wo, bo):
    from concourse.bass_utils import run_bass_kernel_spmd

    nc = get_nc()
    in_maps = make_in_maps(x, Wq, Wk, Wv, Wo, bo)
    res = run_bass_kernel_spmd(nc, in_maps, list(range(NCORES)))
    return combine_outputs(res.results, bo)

